# revision 2
# baseline (speedup 1.0000x reference)
"""Trainium2 Bass kernel for the 3-layer GATv2 network (nn_GAT_35940286333219).

Sharding: nodes contiguously across 8 cores (2048 each); edges partitioned by
destination so segment-softmax/scatter-add stay local; per-layer AllGather of
the source-side transformed features XL = act @ Wl; per-edge source rows via
indirect DMA gather from the gathered table.

v2: bf16 compute (PSUM accumulation fp32), block-batched one-hot builds and
exp, w = xl[src]+xr[dst] computed on the TensorEngine via a paired
(M_T @ XR + I @ xl) PSUM accumulation; per-edge logits use the decomposition
att.lrelu(v) = 0.6*att.v + 0.4*att.|v| with the separable linear term riding
as extra XL/XR weight columns (per-node, not per-edge) and the |.| term via
Abs + weighted row-reduce.  Softmax max-subtraction is skipped (logits are
O(1) for this model; mathematically identical).
"""
import os
import numpy as np
import ml_dtypes

import concourse.bacc as bacc
import concourse.bass as bass
import concourse.mybir as mybir
import concourse.tile as tile
from concourse.bass_utils import run_bass_kernel_spmd
from concourse.masks import make_identity

P = 128
N = 16384
NCORES = 8
NLOC = N // NCORES          # 2048
NBLK = NLOC // P            # 16
F_IN = 128
DIM = 64
HID = 256
FP = mybir.dt.float32
BF = mybir.dt.bfloat16
I32 = mybir.dt.int32
AF = mybir.ActivationFunctionType
ALU = mybir.AluOpType
BF_NP = ml_dtypes.bfloat16

LAST_RESULTS = None


def _prep_edges(edge_index):
    src = np.concatenate([edge_index[0], np.arange(N, dtype=np.int32)]).astype(np.int64)
    dst = np.concatenate([edge_index[1], np.arange(N, dtype=np.int32)]).astype(np.int64)
    order = np.argsort(dst, kind="stable")
    src_s, dst_s = src[order], dst[order]
    blk = dst_s // P
    bc = np.bincount(blk, minlength=NCORES * NBLK)
    NT = int(np.ceil(bc.max() / P))
    EBLK = NT * P
    src_pad = np.zeros((NCORES, NBLK, EBLK), dtype=np.int32)
    dst_pad = np.full((NCORES, NBLK, EBLK), P, dtype=np.float32)   # P = pad marker
    starts = np.concatenate([[0], np.cumsum(bc)])
    for g in range(NCORES * NBLK):
        c, b = divmod(g, NBLK)
        s, e = starts[g], starts[g + 1]
        k = e - s
        src_pad[c, b, :k] = src_s[s:e]
        dst_pad[c, b, :k] = (dst_s[s:e] - g * P).astype(np.float32)
    src_col = src_pad.reshape(NCORES, NBLK, NT, P).transpose(0, 1, 3, 2).copy()
    dst_col = dst_pad.reshape(NCORES, NBLK, NT, P).transpose(0, 1, 3, 2).copy()
    return src_col, dst_col.astype(BF_NP), NT


def _prep_weights(ii):
    out = {}
    for l, H in ((1, 4), (2, 1), (3, 1)):
        Wl = np.asarray(ii[f"Wl{l}"], np.float32)
        Wr = np.asarray(ii[f"Wr{l}"], np.float32)
        att = np.asarray(ii[f"att{l}"], np.float32)
        inD = Wl.shape[0]
        D = HID // H
        attf = att.reshape(-1)
        Wsl = 0.6 * np.stack([(Wl[:, h * D:(h + 1) * D] * att[h][None, :]).sum(1)
                              for h in range(H)], axis=1)
        Wsr = 0.6 * np.stack([(Wr[:, h * D:(h + 1) * D] * att[h][None, :]).sum(1)
                              for h in range(H)], axis=1)
        if H == 1:
            WL = np.concatenate([Wl, np.zeros((inD, 1), np.float32), Wsl], 1)
            WR = np.concatenate([Wr, np.zeros((inD, 1), np.float32), Wsr], 1)
        else:
            WL = np.concatenate([Wl, Wsl], 1)
            WR = np.concatenate([Wr, Wsr], 1)
        out[f"WL{l}"] = WL.astype(BF_NP)
        out[f"WR{l}"] = WR.astype(BF_NP)
        out[f"svec{l}"] = np.tile(0.4 * attf[None, :], (P, 1)).astype(BF_NP)
    return out


def _build(NT):
    nc = bacc.Bacc(None)

    def par(name, shape, dtype=BF):
        return nc.declare_dram_parameter(name, list(shape), dtype, isOutput=False)

    xT = par("xT", [F_IN, NLOC])
    src_col = par("src_col", [NBLK, P, NT], I32)
    dst_col = par("dst_col", [NBLK, P, NT], BF)
    Win = par("Win", [F_IN, DIM]); b_in = par("b_in", [1, DIM])
    Wskip = par("Wskip", [DIM, HID]); bskip = par("bskip", [1, HID])
    WL1 = par("WL1", [DIM, 260]); WR1 = par("WR1", [DIM, 260])
    WL2 = par("WL2", [HID, 258]); WR2 = par("WR2", [HID, 258])
    WL3 = par("WL3", [HID, 258]); WR3 = par("WR3", [HID, 258])
    svec1 = par("svec1", [P, HID]); svec2 = par("svec2", [P, HID]); svec3 = par("svec3", [P, HID])
    Wm1 = par("Wm1", [HID, DIM]); bm1 = par("bm1", [1, DIM])
    Wm2 = par("Wm2", [DIM, DIM]); bm2 = par("bm2", [1, DIM])
    Wm3 = par("Wm3", [DIM, 1]); bm3 = par("bm3", [1, 1])
    iota_f = par("iota_f", [P, P])
    out = nc.declare_dram_parameter("out", [1, NLOC], FP, isOutput=True)

    CL = {1: 260, 2: 258, 3: 258}
    xl_loc = {l: nc.dram_tensor(f"xl_loc{l}", [NLOC, CL[l]], BF) for l in (1, 2, 3)}
    xl_full = {l: nc.dram_tensor(f"xl_full{l}", [N, CL[l]], BF, addr_space="Shared")
               for l in (1, 2, 3)}

    with tile.TileContext(nc) as tc:
        with (
            tc.tile_pool(name="const", bufs=1) as cp,
            tc.tile_pool(name="big", bufs=1) as bigp,
            tc.tile_pool(name="wk", bufs=1) as wk,
            tc.tile_pool(name="ps_mm", bufs=2, space="PSUM") as ps_mm,
            tc.tile_pool(name="ps_out", bufs=2, space="PSUM") as ps_out_pool,
            tc.tile_pool(name="ps_w", bufs=2, space="PSUM") as ps_w_pool,
        ):
            def load_const(pname, ap, shape, dtype=BF):
                t = cp.tile(list(shape), dtype, name=pname + "_sb")
                nc.sync.dma_start(out=t[:], in_=ap[:])
                return t

            def load_const_2k(pname, ap, rows, cols):
                assert rows == 2 * P
                t = cp.tile([P, 2 * cols], BF, name=pname + "_sb")
                nc.sync.dma_start(out=t[:, :cols], in_=ap[:P, :])
                nc.sync.dma_start(out=t[:, cols:], in_=ap[P:, :])
                return t

            ident_f = cp.tile([P, P], FP, name="ident_f")
            make_identity(nc, ident_f[:])
            ident_b = cp.tile([P, P], BF, name="ident_b")
            nc.vector.tensor_copy(out=ident_b[:], in_=ident_f[:])
            ones_row = cp.tile([1, 512], BF, name="ones_row")
            nc.vector.memset(ones_row[:], 1.0)

            xT_sb = load_const("xT", xT, [F_IN, NLOC])
            Win_sb = load_const("Win", Win, [F_IN, DIM])
            b_in_sb = load_const("b_in", b_in, [1, DIM])
            Wskip_sb = load_const("Wskip", Wskip, [DIM, HID])
            bskip_sb = load_const("bskip", bskip, [1, HID])
            WL_sb = {1: load_const("WL1", WL1, [DIM, 260]),
                     2: load_const_2k("WL2", WL2, HID, 258),
                     3: load_const_2k("WL3", WL3, HID, 258)}
            WR_sb = {1: load_const("WR1", WR1, [DIM, 260]),
                     2: load_const_2k("WR2", WR2, HID, 258),
                     3: load_const_2k("WR3", WR3, HID, 258)}
            svec_sb = {1: load_const("svec1", svec1, [P, HID]),
                       2: load_const("svec2", svec2, [P, HID]),
                       3: load_const("svec3", svec3, [P, HID])}
            Wm1_sb = load_const_2k("Wm1", Wm1, HID, DIM)
            bm1_sb = load_const("bm1", bm1, [1, DIM])
            Wm2_sb = load_const("Wm2", Wm2, [DIM, DIM])
            bm2_sb = load_const("bm2", bm2, [1, DIM])
            Wm3_sb = load_const("Wm3", Wm3, [DIM, 1])
            bm3_sb = load_const("bm3", bm3, [1, 1])
            iof_sb = load_const("iota_f", iota_f, [P, P])

            actT = {0: bigp.tile([P, NLOC], BF, name="actT0"),
                    1: bigp.tile([P, NLOC], BF, name="actT1")}
            act_prev = bigp.tile([P, NBLK * HID], FP, name="act_prev")
            act_next = bigp.tile([P, NBLK * HID], FP, name="act_next")
            XRb = bigp.tile([P, NBLK * 260], BF, name="XRb")
            hT = bigp.tile([DIM, NLOC], BF, name="hT")
            m1T = bigp.tile([DIM, NLOC], BF, name="m1T")
            m2T = bigp.tile([DIM, NLOC], BF, name="m2T")
            y_sb = bigp.tile([1, NLOC], FP, name="y_sb")

            # ---------------- phase A ----------------
            for j in range(NLOC // 512):
                sl = slice(j * 512, (j + 1) * 512)
                pmm = ps_mm.tile([P, 512], FP, space="PSUM", name="pmm", tag="pmm")
                nc.tensor.matmul(out=pmm[:DIM, :], lhsT=Win_sb[:], rhs=xT_sb[:, sl],
                                 start=True, stop=False)
                nc.tensor.matmul(out=pmm[:DIM, :], lhsT=b_in_sb[:], rhs=ones_row[:],
                                 start=False, stop=True)
                nc.scalar.activation(out=hT[:DIM, sl], in_=pmm[:DIM, :], func=AF.Relu)

            C1 = CL[1]
            for b in range(NBLK):
                nsl = slice(b * P, (b + 1) * P)
                pxl = ps_mm.tile([P, C1], FP, space="PSUM", name="pxl", tag="pmm")
                nc.tensor.matmul(out=pxl[:], lhsT=hT[:DIM, nsl], rhs=WL_sb[1][:],
                                 start=True, stop=True)
                xl_st = wk.tile([P, C1], BF, name="xl_st", tag="xl_st", bufs=3)
                nc.scalar.activation(out=xl_st[:], in_=pxl[:], func=AF.Copy)
                nc.sync.dma_start(out=xl_loc[1][nsl, :], in_=xl_st[:])

                pxr = ps_mm.tile([P, C1], FP, space="PSUM", name="pxr", tag="pmm")
                nc.tensor.matmul(out=pxr[:], lhsT=hT[:DIM, nsl], rhs=WR_sb[1][:],
                                 start=True, stop=True)
                nc.scalar.activation(out=XRb[:, b * 260:b * 260 + C1], in_=pxr[:], func=AF.Copy)

                psk = ps_mm.tile([P, HID], FP, space="PSUM", name="psk", tag="pmm")
                nc.tensor.matmul(out=psk[:], lhsT=hT[:DIM, nsl], rhs=Wskip_sb[:],
                                 start=True, stop=False)
                nc.tensor.matmul(out=psk[:], lhsT=ones_row[:, :P], rhs=bskip_sb[:],
                                 start=False, stop=True)
                nc.scalar.activation(out=act_prev[:, b * HID:(b + 1) * HID], in_=psk[:],
                                     func=AF.Copy)

            nc.gpsimd.collective_compute(
                "AllGather", ALU.bypass, replica_groups=[list(range(NCORES))],
                ins=[xl_loc[1][:]], outs=[xl_full[1][:]])

            # ---------------- edge stage ----------------
            def edge_layer(l, H):
                C = CL[l]
                D = HID // H
                for b in range(NBLK):
                    src_b = wk.tile([P, NT], I32, name="src_b", tag="src_b", bufs=2)
                    nc.sync.dma_start(out=src_b[:], in_=src_col[b])
                    dst_b = wk.tile([P, NT], BF, name="dst_b", tag="dst_b", bufs=2)
                    nc.sync.dma_start(out=dst_b[:], in_=dst_col[b])
                    m_all = wk.tile([P, NT * P], BF, name="m_all", tag="m_all", bufs=2)
                    nc.vector.tensor_tensor(
                        out=m_all[:].rearrange("p (t e) -> p t e", t=NT),
                        in0=dst_b[:, :, None].to_broadcast([P, NT, P]),
                        in1=iof_sb[:, None, :].to_broadcast([P, NT, P]),
                        op=ALU.is_equal)
                    xl_all = wk.tile([P, NT * C], BF, name="xl_all", tag="xl_all", bufs=2)
                    # one batched gather per block: offsets [P, NT] -> rows [P, NT, C]
                    nc.gpsimd.indirect_dma_start(
                        out=xl_all[:], out_offset=None,
                        in_=xl_full[l][:],
                        in_offset=bass.IndirectOffsetOnAxis(ap=src_b[:], axis=0))
                    r_all = wk.tile([P, NT * H], FP, name="r_all", tag="r_all", bufs=2)
                    alpha_all = wk.tile([P, NT * H], FP, name="alpha_all",
                                        tag="alpha_all", bufs=2)
                    ps_o = ps_out_pool.tile([P, 264], FP, space="PSUM",
                                            name="ps_o", tag="ps_o")
                    for t in range(NT):
                        xl_t = xl_all[:, t * C:(t + 1) * C]
                        m_t = m_all[:, t * P:(t + 1) * P]
                        mt_ps = ps_w_pool.tile([P, P], BF, space="PSUM",
                                               name="mt_ps", tag="mt_ps")
                        nc.tensor.transpose(out=mt_ps[:], in_=m_t, identity=ident_b[:])
                        mt_t = wk.tile([P, P], BF, name="mt_t", tag="mt_t", bufs=4)
                        nc.scalar.activation(out=mt_t[:], in_=mt_ps[:], func=AF.Copy)
                        # w = M @ XR_blk + xl   (PE does the add via PSUM accum)
                        w_ps = ps_w_pool.tile([P, 264], FP, space="PSUM",
                                              name="w_ps", tag="w_ps")
                        nc.tensor.matmul(out=w_ps[:, :C], lhsT=mt_t[:],
                                         rhs=XRb[:, b * 260:b * 260 + C],
                                         start=True, stop=False)
                        nc.tensor.matmul(out=w_ps[:, :C], lhsT=ident_b[:],
                                         rhs=xl_t, start=False, stop=True)
                        a_t = wk.tile([P, HID], BF, name="a_t", tag="a_t", bufs=4)
                        nc.scalar.activation(out=a_t[:], in_=w_ps[:, :HID], func=AF.Abs)
                        off0 = HID + 1 if H == 1 else HID
                        slsr = wk.tile([P, 4], FP, name="slsr", tag="slsr", bufs=4)
                        nc.vector.tensor_scalar(out=slsr[:, :H], in0=w_ps[:, off0:off0 + H],
                                                scalar1=1.0, scalar2=None, op0=ALU.mult)
                        z_t = wk.tile([P, HID], BF, name="z_t", tag="z_t", bufs=2)
                        nc.vector.tensor_tensor(out=z_t[:], in0=a_t[:], in1=svec_sb[l][:],
                                                op=ALU.mult)
                        rr = wk.tile([P, 4], FP, name="rr", tag="rr", bufs=4)
                        nc.vector.tensor_reduce(
                            out=rr[:, :H], in_=z_t[:].rearrange("p (h d) -> p h d", h=H),
                            axis=mybir.AxisListType.X, op=ALU.add)
                        nc.vector.tensor_tensor(out=r_all[:, t * H:(t + 1) * H],
                                                in0=rr[:, :H], in1=slsr[:, :H],
                                                op=ALU.add)
                    nc.scalar.activation(out=alpha_all[:], in_=r_all[:], func=AF.Exp)
                    for t in range(NT):
                        xl_t = xl_all[:, t * C:(t + 1) * C]
                        if H == 1:
                            mp_t = wk.tile([P, P], BF, name="mp_t", tag="mp_t", bufs=4)
                            nc.vector.tensor_scalar(
                                out=mp_t[:], in0=m_all[:, t * P:(t + 1) * P],
                                scalar1=alpha_all[:, t:t + 1], scalar2=None,
                                op0=ALU.mult)
                            nc.tensor.matmul(out=ps_o[:, :257], lhsT=mp_t[:],
                                             rhs=xl_all[:, t * C:t * C + 257],
                                             start=(t == 0), stop=(t == NT - 1))
                        else:
                            v_t = wk.tile([P, HID + 4], BF, name="v_t", tag="v_t", bufs=4)
                            for h in range(H):
                                nc.vector.tensor_scalar(
                                    out=v_t[:, h * D:(h + 1) * D],
                                    in0=xl_all[:, t * C + h * D:t * C + (h + 1) * D],
                                    scalar1=alpha_all[:, t * H + h:t * H + h + 1],
                                    scalar2=None, op0=ALU.mult)
                            nc.vector.tensor_copy(
                                out=v_t[:, HID:HID + H],
                                in_=alpha_all[:, t * H:(t + 1) * H])
                            nc.tensor.matmul(out=ps_o[:, :HID + H], lhsT=m_all[:, t * P:(t + 1) * P],
                                             rhs=v_t[:], start=(t == 0), stop=(t == NT - 1))
                    # finalize
                    rec = wk.tile([P, H], FP, name="rec", tag="rec", bufs=2)
                    nc.vector.reciprocal(out=rec[:], in_=ps_o[:, HID:HID + H])
                    g_t = wk.tile([P, HID], FP, name="g_t", tag="g_t", bufs=2)
                    for h in range(H):
                        hs = slice(h * D, (h + 1) * D)
                        nc.vector.tensor_scalar(out=g_t[:, hs], in0=ps_o[:, hs],
                                                scalar1=rec[:, h:h + 1], scalar2=None,
                                                op0=ALU.mult)
                    gr = wk.tile([P, HID], FP, name="gr", tag="gr", bufs=2)
                    nc.scalar.activation(out=gr[:], in_=g_t[:], func=AF.Relu)
                    bsl = slice(b * HID, (b + 1) * HID)
                    nc.vector.tensor_tensor(out=act_next[:, bsl], in0=gr[:],
                                            in1=act_prev[:, bsl], op=ALU.add)
                    for k in range(2):
                        tp = ps_mm.tile([P, P], FP, space="PSUM", name="tp", tag="pmm")
                        nc.tensor.transpose(
                            out=tp[:], in_=act_next[:, b * HID + k * P:b * HID + (k + 1) * P],
                            identity=ident_f[:])
                        nc.scalar.activation(out=actT[k][:, b * P:(b + 1) * P],
                                             in_=tp[:], func=AF.Copy)

            _ocr = {}

            def _ones_col_row(l):
                if l not in _ocr:
                    t = cp.tile([1, CL[l]], BF, name=f"onescol{l}")
                    nc.vector.memset(t[:], 0.0)
                    nc.vector.memset(t[:, HID:HID + 1], 1.0)
                    _ocr[l] = t
                return _ocr[l][:]

            def xlxr_layer(l):
                C = CL[l]
                for b in range(NBLK):
                    nsl = slice(b * P, (b + 1) * P)
                    pxl = ps_mm.tile([P, C], FP, space="PSUM", name="pxl2", tag="pmm")
                    for k in range(2):
                        nc.tensor.matmul(out=pxl[:], lhsT=actT[k][:, nsl],
                                         rhs=WL_sb[l][:, k * C:(k + 1) * C],
                                         start=(k == 0), stop=False)
                    nc.tensor.matmul(out=pxl[:], lhsT=ones_row[:, :P],
                                     rhs=_ones_col_row(l), start=False, stop=True)
                    xl_st = wk.tile([P, C], BF, name="xl_st2", tag="xl_st", bufs=3)
                    nc.scalar.activation(out=xl_st[:], in_=pxl[:], func=AF.Copy)
                    nc.sync.dma_start(out=xl_loc[l][nsl, :], in_=xl_st[:])

                    pxr = ps_mm.tile([P, C], FP, space="PSUM", name="pxr2", tag="pmm")
                    for k in range(2):
                        nc.tensor.matmul(out=pxr[:], lhsT=actT[k][:, nsl],
                                         rhs=WR_sb[l][:, k * C:(k + 1) * C],
                                         start=(k == 0), stop=(k == 1))
                    nc.scalar.activation(out=XRb[:, b * 260:b * 260 + C], in_=pxr[:],
                                         func=AF.Copy)
                nc.gpsimd.collective_compute(
                    "AllGather", ALU.bypass, replica_groups=[list(range(NCORES))],
                    ins=[xl_loc[l][:]], outs=[xl_full[l][:]])

            edge_layer(1, 4)
            act_prev, act_next = act_next, act_prev
            xlxr_layer(2)
            edge_layer(2, 1)
            act_prev, act_next = act_next, act_prev
            xlxr_layer(3)
            edge_layer(3, 1)

            # ---------------- MLP head ----------------
            for j in range(NLOC // 512):
                sl = slice(j * 512, (j + 1) * 512)
                pm1 = ps_mm.tile([P, 512], FP, space="PSUM", name="pm1", tag="pmm")
                for k in range(2):
                    nc.tensor.matmul(out=pm1[:DIM, :], lhsT=Wm1_sb[:, k * DIM:(k + 1) * DIM],
                                     rhs=actT[k][:, sl], start=(k == 0), stop=False)
                nc.tensor.matmul(out=pm1[:DIM, :], lhsT=bm1_sb[:], rhs=ones_row[:],
                                 start=False, stop=True)
                nc.scalar.activation(out=m1T[:DIM, sl], in_=pm1[:DIM, :], func=AF.Relu)
            for j in range(NLOC // 512):
                sl = slice(j * 512, (j + 1) * 512)
                pm2 = ps_mm.tile([P, 512], FP, space="PSUM", name="pm2", tag="pmm")
                nc.tensor.matmul(out=pm2[:DIM, :], lhsT=Wm2_sb[:], rhs=m1T[:DIM, sl],
                                 start=True, stop=False)
                nc.tensor.matmul(out=pm2[:DIM, :], lhsT=bm2_sb[:], rhs=ones_row[:],
                                 start=False, stop=True)
                nc.scalar.activation(out=m2T[:DIM, sl], in_=pm2[:DIM, :], func=AF.Relu)
            for j in range(NLOC // 512):
                sl = slice(j * 512, (j + 1) * 512)
                py = ps_mm.tile([P, 512], FP, space="PSUM", name="py", tag="pmm")
                nc.tensor.matmul(out=py[:1, :], lhsT=Wm3_sb[:], rhs=m2T[:DIM, sl],
                                 start=True, stop=False)
                nc.tensor.matmul(out=py[:1, :], lhsT=bm3_sb[:], rhs=ones_row[:],
                                 start=False, stop=True)
                nc.scalar.activation(out=y_sb[:, sl], in_=py[:1, :], func=AF.Copy)
            nc.sync.dma_start(out=out[:], in_=y_sb[:])

    nc.compile()
    return nc


_BUILD_CACHE = {}


def _get_program(NT):
    if NT not in _BUILD_CACHE:
        _BUILD_CACHE[NT] = _build(NT)
    return _BUILD_CACHE[NT]


def kernel(**inputs) -> np.ndarray:
    global LAST_RESULTS
    ii = {k: np.asarray(v) for k, v in inputs.items()}
    assert ii["x"].shape == (N, F_IN)
    for l in (1, 2, 3):
        assert not np.any(ii[f"b{l}"]), "GAT bias assumed zero"

    src_col, dst_col, NT = _prep_edges(np.asarray(ii["edge_index"], np.int64))
    w = _prep_weights(ii)
    iota = np.arange(P, dtype=BF_NP)
    iota_f = np.tile(iota[None, :], (P, 1))

    def bf(a):
        return np.asarray(a, np.float32).astype(BF_NP)

    common = dict(
        Win=bf(ii["Win"]), b_in=bf(ii["b_in"])[None, :],
        Wskip=bf(ii["Wskip"]), bskip=bf(ii["bskip"])[None, :],
        WL1=w["WL1"], WR1=w["WR1"], svec1=w["svec1"],
        WL2=w["WL2"], WR2=w["WR2"], svec2=w["svec2"],
        WL3=w["WL3"], WR3=w["WR3"], svec3=w["svec3"],
        Wm1=bf(ii["Wm1"]), bm1=bf(ii["bm1"])[None, :],
        Wm2=bf(ii["Wm2"]), bm2=bf(ii["bm2"])[None, :],
        Wm3=bf(ii["Wm3"]), bm3=bf(ii["bm3"])[None, :],
        iota_f=iota_f,
    )
    x = np.asarray(ii["x"], np.float32)
    in_maps = []
    for c in range(NCORES):
        m = dict(common)
        m["xT"] = np.ascontiguousarray(x[c * NLOC:(c + 1) * NLOC].T).astype(BF_NP)
        m["src_col"] = src_col[c]
        m["dst_col"] = dst_col[c]
        in_maps.append(m)

    nc = _get_program(NT)
    res = run_bass_kernel_spmd(nc, in_maps, list(range(NCORES)),
                               trace=bool(os.environ.get("GAT_TRACE")))
    LAST_RESULTS = res
    return np.concatenate([res.results[c]["out"].reshape(-1) for c in range(NCORES)])



# revision 15
# speedup vs baseline: 1.0522x; 1.0522x over previous
"""Trainium2 Bass kernel for the 3-layer GATv2 network (nn_GAT_35940286333219).

Sharding: nodes across 8 cores (2048 each) after a degree-balancing
permutation (every 128-node dst block gets an equal edge count -> minimal
padding NT); edges partitioned by destination block so segment-softmax and
scatter-add stay local; per-layer AllGather of XL = act @ Wl; per-edge source
rows fetched with one batched dma_gather per block (512B rows).

v5:
  - dma_gather (InstDMAGatherAnt) per block instead of 17 per-t indirect DMAs.
  - logits: ACT Prelu(alpha=0.2) pass over a 2-tile PSUM bank pair, then
    att-weighted DVE mult + segmented reduce. Gathered rows are exactly 256
    cols (512B).
  - M (edge-major one-hot, scatter lhsT) built in DVE 2x mode from a
    pre-expanded dst tensor; M^T (xr-gather lhsT) built with a per-partition
    iota tensor_scalar from a pre-replicated dst row. No PE transposes.
  - alpha kept in bf16: feeds tensor_scalar scaling and the denominator
    matmuls directly (no casts).
  - degree-balanced node relabeling: NT = 17 with zero padding waste.
"""
import os
import numpy as np
import ml_dtypes

import concourse.bacc as bacc
import concourse.bass as bass
import concourse.mybir as mybir
import concourse.tile as tile
from concourse.bass_utils import run_bass_kernel_spmd
from concourse.masks import make_identity

P = 128
N = 16384
NCORES = 8
NLOC = N // NCORES          # 2048
NBLK = NLOC // P            # 16
F_IN = 128
DIM = 64
HID = 256
FP = mybir.dt.float32
BF = mybir.dt.bfloat16
I16 = mybir.dt.int16
AF = mybir.ActivationFunctionType
ALU = mybir.AluOpType
BF_NP = ml_dtypes.bfloat16
NEG = 0.2

LAST_RESULTS = None


def _balance_nodes(edge_index):
    """Permute node ids so every 128-node block has ~equal in-degree sum."""
    import heapq
    deg = np.bincount(edge_index[1], minlength=N).astype(np.int64) + 1  # + self loop
    order = np.argsort(-deg, kind="stable")
    nblk_g = N // P
    heap = [(0, g) for g in range(nblk_g)]
    heapq.heapify(heap)
    counts = np.zeros(nblk_g, np.int64)
    loads = np.zeros(nblk_g, np.int64)
    assign = np.empty(N, np.int64)
    for v in order:
        while True:
            load, g = heapq.heappop(heap)
            if counts[g] < P:
                break
        assign[v] = g
        counts[g] += 1
        loads[g] = load + deg[v]
        if counts[g] < P:
            heapq.heappush(heap, (loads[g], g))
    # repair: shave overloaded blocks down to the ceiling with 1-for-1 swaps
    # of nodes whose degrees differ by exactly the overload delta (usually 1)
    cap = int(np.ceil(deg.sum() / nblk_g))
    by_block = [list(np.where(assign == g)[0]) for g in range(nblk_g)]
    for _ in range(64):
        over = [g for g in range(nblk_g) if loads[g] > cap]
        if not over:
            break
        under = sorted((g for g in range(nblk_g) if loads[g] < cap),
                       key=lambda g: loads[g])
        for go in over:
            d = int(loads[go] - cap)
            done = False
            for gu in under:
                if loads[gu] + d > cap:
                    continue
                du = {deg[v]: v for v in by_block[gu]}
                for a in by_block[go]:
                    b = du.get(deg[a] - d)
                    if b is not None:
                        assign[a], assign[b] = gu, go
                        by_block[go].remove(a); by_block[gu].remove(b)
                        by_block[go].append(b); by_block[gu].append(a)
                        loads[go] -= d; loads[gu] += d
                        done = True
                        break
                if done:
                    break
    perm = np.argsort(assign, kind="stable")      # new -> old
    rank = np.empty(N, np.int64)
    rank[perm] = np.arange(N)                     # old -> new
    return perm, rank


def _prep_edges(src, dst):
    """src/dst already relabeled. Returns gather indices + dst tables."""
    order = np.argsort(dst, kind="stable")
    src_s, dst_s = src[order], dst[order]
    blk = dst_s // P
    bc = np.bincount(blk, minlength=NCORES * NBLK)
    NT = int(np.ceil(bc.max() / P))
    EBLK = NT * P
    src_pad = np.zeros((NCORES, NBLK, EBLK), dtype=np.int32)
    dst_pad = np.full((NCORES, NBLK, EBLK), P, dtype=np.float32)   # P = pad marker
    starts = np.concatenate([[0], np.cumsum(bc)])
    for g in range(NCORES * NBLK):
        c, b = divmod(g, NBLK)
        s, e = starts[g], starts[g + 1]
        k = e - s
        src_pad[c, b, :k] = src_s[s:e]
        dst_pad[c, b, :k] = (dst_s[s:e] - g * P).astype(np.float32)
    # dma_gather idx layout: flat[i] = idxs[i % 16, i // 16], 16-row pattern
    # replicated down 128 partitions
    idx16 = (src_pad.reshape(NCORES, NBLK, EBLK // 16, 16)
             .transpose(0, 1, 3, 2).astype(np.int16))
    idx16 = np.tile(idx16, (1, 1, 8, 1)).copy()        # [NC, NBLK, 128, EBLK/16]
    # slot (p, t) = flat index t*P + p
    dst_col = dst_pad.reshape(NCORES, NBLK, NT, P).transpose(0, 1, 3, 2)
    # dstpack: [P, NT*P + NT*P] =
    #   [dstrep (dst replicated across partitions, for the MT build)
    #    | dst_exp (dst value repeated along d, for the 2x-mode M build)]
    dstpack = np.empty((NCORES, NBLK, P, NT * 2 * P), dtype=BF_NP)
    dstpack[:, :, :, :EBLK] = np.broadcast_to(
        dst_pad[:, :, None, :], (NCORES, NBLK, P, EBLK)).astype(BF_NP)
    dstpack[:, :, :, EBLK:] = np.repeat(dst_col, P, axis=3).astype(BF_NP)
    return idx16, dstpack, NT


def _build(NT):
    nc = bacc.Bacc(None)
    NPAIR = (NT + 1) // 2

    def par(name, shape, dtype=BF):
        return nc.declare_dram_parameter(name, list(shape), dtype, isOutput=False)

    xT = par("xT", [F_IN, NLOC])
    idx16 = par("idx16", [NBLK, P, NT * 8], I16)
    dstpack = par("dstpack", [NBLK, P, NT * 2 * P])
    Win = par("Win", [F_IN, DIM]); b_in = par("b_in", [1, DIM])
    Wskip = par("Wskip", [DIM, HID]); bskip = par("bskip", [1, HID])
    WL1 = par("WL1", [DIM, HID]); WR1 = par("WR1", [DIM, HID])
    WL2 = par("WL2", [HID, HID]); WR2 = par("WR2", [HID, HID])
    WL3 = par("WL3", [HID, HID]); WR3 = par("WR3", [HID, HID])
    attv1 = par("attv1", [P, 2 * HID]); attv2 = par("attv2", [P, 2 * HID])
    attv3 = par("attv3", [P, 2 * HID])
    Wm1 = par("Wm1", [HID, DIM]); bm1 = par("bm1", [1, DIM])
    Wm2 = par("Wm2", [DIM, DIM]); bm2 = par("bm2", [1, DIM])
    Wm3 = par("Wm3", [DIM, 1]); bm3 = par("bm3", [1, 1])
    iota_row = par("iota_row", [P, P])
    iota_col = par("iota_col", [P, 1], FP)
    out = nc.declare_dram_parameter("out", [1, NLOC], FP, isOutput=True)

    xl_loc = {l: nc.dram_tensor(f"xl_loc{l}", [NLOC, HID], BF) for l in (1, 2, 3)}
    xl_full = {l: nc.dram_tensor(f"xl_full{l}", [N, HID], BF, addr_space="Shared")
               for l in (1, 2, 3)}

    with tile.TileContext(nc) as tc:
        with (
            tc.tile_pool(name="const", bufs=1) as cp,
            tc.tile_pool(name="big", bufs=1) as bigp,
            tc.tile_pool(name="wk", bufs=1) as wk,
            tc.tile_pool(name="ps_mm", bufs=2, space="PSUM") as ps_mm,
            tc.tile_pool(name="ps_out", bufs=2, space="PSUM") as ps_out_pool,
            tc.tile_pool(name="ps_w", bufs=3, space="PSUM") as ps_w_pool,
        ):
            def load_const(pname, ap, shape, dtype=BF):
                t = cp.tile(list(shape), dtype, name=pname + "_sb")
                nc.sync.dma_start(out=t[:], in_=ap[:])
                return t

            def load_const_2k(pname, ap, rows, cols):
                assert rows == 2 * P
                t = cp.tile([P, 2 * cols], BF, name=pname + "_sb")
                nc.sync.dma_start(out=t[:, :cols], in_=ap[:P, :])
                nc.sync.dma_start(out=t[:, cols:], in_=ap[P:, :])
                return t

            ident_f = cp.tile([P, P], FP, name="ident_f")
            make_identity(nc, ident_f[:])
            ident_b = cp.tile([P, P], BF, name="ident_b")
            nc.vector.tensor_copy(out=ident_b[:], in_=ident_f[:])
            ones_row = cp.tile([1, 512], BF, name="ones_row")
            nc.vector.memset(ones_row[:], 1.0)
            ones_col = cp.tile([P, 1], BF, name="ones_col")
            nc.vector.memset(ones_col[:], 1.0)

            xT_sb = load_const("xT", xT, [F_IN, NLOC])
            Win_sb = load_const("Win", Win, [F_IN, DIM])
            b_in_sb = load_const("b_in", b_in, [1, DIM])
            Wskip_sb = load_const("Wskip", Wskip, [DIM, HID])
            bskip_sb = load_const("bskip", bskip, [1, HID])
            WL_sb = {1: load_const("WL1", WL1, [DIM, HID]),
                     2: load_const_2k("WL2", WL2, HID, HID),
                     3: load_const_2k("WL3", WL3, HID, HID)}
            WR_sb = {1: load_const("WR1", WR1, [DIM, HID]),
                     2: load_const_2k("WR2", WR2, HID, HID),
                     3: load_const_2k("WR3", WR3, HID, HID)}
            attv_sb = {1: load_const("attv1", attv1, [P, 2 * HID]),
                       2: load_const("attv2", attv2, [P, 2 * HID]),
                       3: load_const("attv3", attv3, [P, 2 * HID])}
            Wm1_sb = load_const_2k("Wm1", Wm1, HID, DIM)
            bm1_sb = load_const("bm1", bm1, [1, DIM])
            Wm2_sb = load_const("Wm2", Wm2, [DIM, DIM])
            bm2_sb = load_const("bm2", bm2, [1, DIM])
            Wm3_sb = load_const("Wm3", Wm3, [DIM, 1])
            bm3_sb = load_const("bm3", bm3, [1, 1])
            ior_sb = load_const("iota_row", iota_row, [P, P])
            ioc_sb = load_const("iota_col", iota_col, [P, 1], FP)

            actT = {0: bigp.tile([P, NLOC], BF, name="actT0"),
                    1: bigp.tile([P, NLOC], BF, name="actT1")}
            act_prev = bigp.tile([P, NBLK * HID], FP, name="act_prev")
            act_next = bigp.tile([P, NBLK * HID], FP, name="act_next")
            XRb = bigp.tile([P, NBLK * HID], BF, name="XRb")
            hT = bigp.tile([DIM, NLOC], BF, name="hT")
            m1T = bigp.tile([DIM, NLOC], BF, name="m1T")
            m2T = bigp.tile([DIM, NLOC], BF, name="m2T")
            y_sb = bigp.tile([1, NLOC], FP, name="y_sb")

            # ---------------- phase A: h = relu(x @ Win + b) ----------------
            for j in range(NLOC // 512):
                sl = slice(j * 512, (j + 1) * 512)
                pmm = ps_mm.tile([P, 512], FP, space="PSUM", name="pmm", tag="pmm")
                nc.tensor.matmul(out=pmm[:DIM, :], lhsT=Win_sb[:], rhs=xT_sb[:, sl],
                                 start=True, stop=False)
                nc.tensor.matmul(out=pmm[:DIM, :], lhsT=b_in_sb[:], rhs=ones_row[:],
                                 start=False, stop=True)
                nc.scalar.activation(out=hT[:DIM, sl], in_=pmm[:DIM, :], func=AF.Relu)

            # layer-1 XL/XR + skip
            for b in range(NBLK):
                nsl = slice(b * P, (b + 1) * P)
                pxl = ps_mm.tile([P, HID], FP, space="PSUM", name="pxl", tag="pmm")
                nc.tensor.matmul(out=pxl[:], lhsT=hT[:DIM, nsl], rhs=WL_sb[1][:],
                                 start=True, stop=True)
                xl_st = wk.tile([P, HID], BF, name="xl_st", tag="xl_st", bufs=3)
                nc.scalar.activation(out=xl_st[:], in_=pxl[:], func=AF.Copy)
                nc.sync.dma_start(out=xl_loc[1][nsl, :], in_=xl_st[:])

                pxr = ps_mm.tile([P, HID], FP, space="PSUM", name="pxr", tag="pmm")
                nc.tensor.matmul(out=pxr[:], lhsT=hT[:DIM, nsl], rhs=WR_sb[1][:],
                                 start=True, stop=True)
                nc.scalar.activation(out=XRb[:, b * HID:(b + 1) * HID], in_=pxr[:],
                                     func=AF.Copy)

                psk = ps_mm.tile([P, HID], FP, space="PSUM", name="psk", tag="pmm")
                nc.tensor.matmul(out=psk[:], lhsT=hT[:DIM, nsl], rhs=Wskip_sb[:],
                                 start=True, stop=False)
                nc.tensor.matmul(out=psk[:], lhsT=ones_row[:, :P], rhs=bskip_sb[:],
                                 start=False, stop=True)
                nc.scalar.activation(out=act_prev[:, b * HID:(b + 1) * HID], in_=psk[:],
                                     func=AF.Copy)

            nc.gpsimd.collective_compute(
                "AllGather", ALU.bypass, replica_groups=[list(range(NCORES))],
                ins=[xl_loc[1][:]], outs=[xl_full[1][:]])

            # ---------------- edge stage ----------------
            def edge_layer(l, H):
                D = HID // H
                for b in range(NBLK):
                    idx_sb = wk.tile([P, NT * 8], I16, name="idx_sb",
                                     tag="idx_sb", bufs=2)
                    nc.sync.dma_start(out=idx_sb[:], in_=idx16[b])
                    dpk = wk.tile([P, NT * 2 * P], BF, name="dpk", tag="dpk", bufs=2)
                    nc.sync.dma_start(out=dpk[:], in_=dstpack[b])
                    drep = dpk[:, :NT * P]
                    dexp = dpk[:, NT * P:]
                    # edge-major one-hot M[e, (t,d)]  (both operands unit-stride)
                    M = wk.tile([P, NT * P], BF, name="M", tag="M", bufs=2)
                    nc.vector.tensor_tensor(
                        out=M[:].rearrange("p (t e) -> p t e", t=NT),
                        in0=dexp[:].rearrange("p (t e) -> p t e", t=NT),
                        in1=ior_sb[:, None, :].to_broadcast([P, NT, P]),
                        op=ALU.is_equal)
                    # dst-major one-hot MT[d, (t,e)]
                    MT = wk.tile([P, NT * P], BF, name="MT", tag="MT", bufs=2)
                    nc.vector.tensor_scalar(out=MT[:], in0=drep,
                                            scalar1=ioc_sb[:, :1], scalar2=None,
                                            op0=ALU.is_equal)
                    xl_all = wk.tile([P, NT * HID], BF, name="xl_all", tag="xl_all",
                                     bufs=2)
                    nc.gpsimd.dma_gather(
                        out_ap=xl_all[:].rearrange("p (t c) -> p t c", t=NT),
                        in_ap=xl_full[l][:],
                        idxs_ap=idx_sb[:],
                        num_idxs=NT * P, num_idxs_reg=NT * P,
                        elem_size=HID, single_packet=False)

                    r_all = wk.tile([P, NT * H], FP, name="r_all", tag="r_all", bufs=2)
                    for j in range(NPAIR):
                        t0 = 2 * j
                        two = t0 + 1 < NT
                        W = 512 if two else 256
                        wps = ps_w_pool.tile([P, 512], FP, space="PSUM",
                                             name="wps", tag="wps")
                        nc.tensor.matmul(out=wps[:, :HID],
                                         lhsT=MT[:, t0 * P:(t0 + 1) * P],
                                         rhs=XRb[:, b * HID:(b + 1) * HID],
                                         start=True, stop=False)
                        if two:
                            # start=False: the bank-wide has_written clear
                            # already happened; unset bits => plain overwrite
                            nc.tensor.matmul(out=wps[:, HID:2 * HID],
                                             lhsT=MT[:, (t0 + 1) * P:(t0 + 2) * P],
                                             rhs=XRb[:, b * HID:(b + 1) * HID],
                                             start=False, stop=False)
                        nc.tensor.matmul(out=wps[:, :W], lhsT=ident_b[:],
                                         rhs=xl_all[:, t0 * HID:t0 * HID + W],
                                         start=False, stop=True)
                        e_p = wk.tile([P, 512], BF, name="e_p", tag="e_p", bufs=3)
                        nc.scalar.activation(out=e_p[:, :W], in_=wps[:, :W],
                                             func=AF.Prelu, alpha=NEG)
                        z_p = wk.tile([P, 512], BF, name="z_p", tag="z_p", bufs=3)
                        nc.vector.tensor_tensor(out=z_p[:, :W], in0=e_p[:, :W],
                                                in1=attv_sb[l][:, :W], op=ALU.mult)
                        u = (2 * H) if two else H
                        nc.vector.tensor_reduce(
                            out=r_all[:, t0 * H:t0 * H + u],
                            in_=z_p[:, :W].rearrange("p (u d) -> p u d", d=D),
                            axis=mybir.AxisListType.X, op=ALU.add)

                    alpha_all = wk.tile([P, NT * H], FP, name="alpha_all",
                                        tag="alpha_all", bufs=2)
                    nc.scalar.activation(out=alpha_all[:], in_=r_all[:], func=AF.Exp)
                    if H > 1:
                        alpha_bf = wk.tile([P, NT * H], BF, name="alpha_bf",
                                           tag="alpha_bf", bufs=2)
                        nc.vector.tensor_copy(out=alpha_bf[:], in_=alpha_all[:])

                    ps_o = ps_out_pool.tile([P, 264], FP, space="PSUM",
                                            name="ps_o", tag="ps_o")
                    if H == 1:
                        for t in range(NT):
                            mp = wk.tile([P, P], BF, name="mp", tag="mp", bufs=4)
                            nc.vector.tensor_scalar(
                                out=mp[:], in0=M[:, t * P:(t + 1) * P],
                                scalar1=alpha_all[:, t:t + 1], scalar2=None,
                                op0=ALU.mult)
                            nc.tensor.matmul(out=ps_o[:, :HID], lhsT=mp[:],
                                             rhs=xl_all[:, t * HID:(t + 1) * HID],
                                             start=(t == 0), stop=(t == NT - 1))
                            nc.tensor.matmul(out=ps_o[:, HID:HID + 1], lhsT=mp[:],
                                             rhs=ones_col[:],
                                             start=False, stop=(t == NT - 1))
                    else:
                        for t in range(NT):
                            v = wk.tile([P, HID], BF, name="v", tag="v", bufs=4)
                            nc.vector.tensor_tensor(
                                out=v[:].rearrange("p (h d) -> p h d", h=H),
                                in0=xl_all[:, t * HID:(t + 1) * HID].rearrange(
                                    "p (h d) -> p h d", h=H),
                                in1=alpha_bf[:, t * H:(t + 1) * H, None].to_broadcast(
                                    [P, H, D]),
                                op=ALU.mult)
                            nc.tensor.matmul(out=ps_o[:, :HID],
                                             lhsT=M[:, t * P:(t + 1) * P],
                                             rhs=v[:],
                                             start=(t == 0), stop=(t == NT - 1))
                            nc.tensor.matmul(out=ps_o[:, HID:HID + H],
                                             lhsT=M[:, t * P:(t + 1) * P],
                                             rhs=alpha_bf[:, t * H:(t + 1) * H],
                                             start=False, stop=(t == NT - 1))
                    # finalize
                    rec = wk.tile([P, H], FP, name="rec", tag="rec", bufs=2)
                    nc.vector.reciprocal(out=rec[:], in_=ps_o[:, HID:HID + H])
                    g_t = wk.tile([P, HID], FP, name="g_t", tag="g_t", bufs=2)
                    for h in range(H):
                        hs = slice(h * D, (h + 1) * D)
                        nc.vector.tensor_scalar(out=g_t[:, hs], in0=ps_o[:, hs],
                                                scalar1=rec[:, h:h + 1], scalar2=None,
                                                op0=ALU.mult)
                    gr = wk.tile([P, HID], BF, name="gr", tag="gr", bufs=2)
                    nc.scalar.activation(out=gr[:], in_=g_t[:], func=AF.Relu)
                    bsl = slice(b * HID, (b + 1) * HID)
                    nc.vector.tensor_tensor(out=act_next[:, bsl], in0=gr[:],
                                            in1=act_prev[:, bsl], op=ALU.add)
                    for k in range(2):
                        tp = ps_mm.tile([P, P], FP, space="PSUM", name="tp", tag="pmm")
                        nc.tensor.transpose(
                            out=tp[:],
                            in_=act_next[:, b * HID + k * P:b * HID + (k + 1) * P],
                            identity=ident_f[:])
                        nc.scalar.activation(out=actT[k][:, b * P:(b + 1) * P],
                                             in_=tp[:], func=AF.Copy)

            def xlxr_layer(l):
                for b in range(NBLK):
                    nsl = slice(b * P, (b + 1) * P)
                    pxl = ps_mm.tile([P, HID], FP, space="PSUM", name="pxl2", tag="pmm")
                    for k in range(2):
                        nc.tensor.matmul(out=pxl[:], lhsT=actT[k][:, nsl],
                                         rhs=WL_sb[l][:, k * HID:(k + 1) * HID],
                                         start=(k == 0), stop=(k == 1))
                    xl_st = wk.tile([P, HID], BF, name="xl_st2", tag="xl_st", bufs=3)
                    nc.scalar.activation(out=xl_st[:], in_=pxl[:], func=AF.Copy)
                    nc.sync.dma_start(out=xl_loc[l][nsl, :], in_=xl_st[:])

                    pxr = ps_mm.tile([P, HID], FP, space="PSUM", name="pxr2", tag="pmm")
                    for k in range(2):
                        nc.tensor.matmul(out=pxr[:], lhsT=actT[k][:, nsl],
                                         rhs=WR_sb[l][:, k * HID:(k + 1) * HID],
                                         start=(k == 0), stop=(k == 1))
                    nc.scalar.activation(out=XRb[:, b * HID:(b + 1) * HID], in_=pxr[:],
                                         func=AF.Copy)
                nc.gpsimd.collective_compute(
                    "AllGather", ALU.bypass, replica_groups=[list(range(NCORES))],
                    ins=[xl_loc[l][:]], outs=[xl_full[l][:]])

            edge_layer(1, 4)
            act_prev, act_next = act_next, act_prev
            xlxr_layer(2)
            edge_layer(2, 1)
            act_prev, act_next = act_next, act_prev
            xlxr_layer(3)
            edge_layer(3, 1)

            # ---------------- MLP head ----------------
            for j in range(NLOC // 512):
                sl = slice(j * 512, (j + 1) * 512)
                pm1 = ps_mm.tile([P, 512], FP, space="PSUM", name="pm1", tag="pmm")
                for k in range(2):
                    nc.tensor.matmul(out=pm1[:DIM, :],
                                     lhsT=Wm1_sb[:, k * DIM:(k + 1) * DIM],
                                     rhs=actT[k][:, sl], start=(k == 0), stop=False)
                nc.tensor.matmul(out=pm1[:DIM, :], lhsT=bm1_sb[:], rhs=ones_row[:],
                                 start=False, stop=True)
                nc.scalar.activation(out=m1T[:DIM, sl], in_=pm1[:DIM, :], func=AF.Relu)
            for j in range(NLOC // 512):
                sl = slice(j * 512, (j + 1) * 512)
                pm2 = ps_mm.tile([P, 512], FP, space="PSUM", name="pm2", tag="pmm")
                nc.tensor.matmul(out=pm2[:DIM, :], lhsT=Wm2_sb[:], rhs=m1T[:DIM, sl],
                                 start=True, stop=False)
                nc.tensor.matmul(out=pm2[:DIM, :], lhsT=bm2_sb[:], rhs=ones_row[:],
                                 start=False, stop=True)
                nc.scalar.activation(out=m2T[:DIM, sl], in_=pm2[:DIM, :], func=AF.Relu)
            for j in range(NLOC // 512):
                sl = slice(j * 512, (j + 1) * 512)
                py = ps_mm.tile([P, 512], FP, space="PSUM", name="py", tag="pmm")
                nc.tensor.matmul(out=py[:1, :], lhsT=Wm3_sb[:], rhs=m2T[:DIM, sl],
                                 start=True, stop=False)
                nc.tensor.matmul(out=py[:1, :], lhsT=bm3_sb[:], rhs=ones_row[:],
                                 start=False, stop=True)
                nc.scalar.activation(out=y_sb[:, sl], in_=py[:1, :], func=AF.Copy)
            nc.sync.dma_start(out=out[:], in_=y_sb[:])

    nc.compile()
    return nc


_BUILD_CACHE = {}


def _get_program(NT):
    if NT not in _BUILD_CACHE:
        _BUILD_CACHE[NT] = _build(NT)
    return _BUILD_CACHE[NT]


def kernel(**inputs) -> np.ndarray:
    global LAST_RESULTS
    ii = {k: np.asarray(v) for k, v in inputs.items()}
    assert ii["x"].shape == (N, F_IN)
    for l in (1, 2, 3):
        assert not np.any(ii[f"b{l}"]), "GAT bias assumed zero"

    ei = np.asarray(ii["edge_index"], np.int64)
    perm, rank = _balance_nodes(ei)
    src = np.concatenate([rank[ei[0]], np.arange(N, dtype=np.int64)])
    dst = np.concatenate([rank[ei[1]], np.arange(N, dtype=np.int64)])
    idx16, dstpack, NT = _prep_edges(src, dst)

    def bf(a):
        return np.asarray(a, np.float32).astype(BF_NP)

    att_row = {}
    for l in (1, 2, 3):
        a = np.asarray(ii[f"att{l}"], np.float32).reshape(-1)  # [HID]
        att_row[l] = np.tile(np.concatenate([a, a])[None, :], (P, 1)).astype(BF_NP)

    iota = np.arange(P, dtype=np.float32)
    common = dict(
        Win=bf(ii["Win"]), b_in=bf(ii["b_in"])[None, :],
        Wskip=bf(ii["Wskip"]), bskip=bf(ii["bskip"])[None, :],
        WL1=bf(ii["Wl1"]), WR1=bf(ii["Wr1"]), attv1=att_row[1],
        WL2=bf(ii["Wl2"]), WR2=bf(ii["Wr2"]), attv2=att_row[2],
        WL3=bf(ii["Wl3"]), WR3=bf(ii["Wr3"]), attv3=att_row[3],
        Wm1=bf(ii["Wm1"]), bm1=bf(ii["bm1"])[None, :],
        Wm2=bf(ii["Wm2"]), bm2=bf(ii["bm2"])[None, :],
        Wm3=bf(ii["Wm3"]), bm3=bf(ii["bm3"])[None, :],
        iota_row=np.tile(iota[None, :], (P, 1)).astype(BF_NP),
        iota_col=iota[:, None].copy(),
    )
    x = np.asarray(ii["x"], np.float32)[perm]
    in_maps = []
    for c in range(NCORES):
        m = dict(common)
        m["xT"] = np.ascontiguousarray(x[c * NLOC:(c + 1) * NLOC].T).astype(BF_NP)
        m["idx16"] = idx16[c]
        m["dstpack"] = dstpack[c]
        in_maps.append(m)

    nc = _get_program(NT)
    res = run_bass_kernel_spmd(nc, in_maps, list(range(NCORES)),
                               trace=bool(os.environ.get("GAT_TRACE")))
    LAST_RESULTS = res
    y_new = np.concatenate([res.results[c]["out"].reshape(-1) for c in range(NCORES)])
    y = np.empty(N, np.float32)
    y[perm] = y_new
    return y


# revision 16
# speedup vs baseline: 1.1013x; 1.0467x over previous
"""Trainium2 Bass kernel for the 3-layer GATv2 network (nn_GAT_35940286333219).

Sharding: nodes across 8 cores (2048 each) after a degree-balancing
permutation (every 128-node dst block gets an equal edge count -> minimal
padding NT); edges partitioned by destination block so segment-softmax and
scatter-add stay local; per-layer AllGather of XL = act @ Wl; per-edge source
rows fetched with one batched dma_gather per block (512B rows).

v5:
  - dma_gather (InstDMAGatherAnt) per block instead of 17 per-t indirect DMAs.
  - logits: ACT Prelu(alpha=0.2) pass over a 2-tile PSUM bank pair, then
    att-weighted DVE mult + segmented reduce. Gathered rows are exactly 256
    cols (512B).
  - M (edge-major one-hot, scatter lhsT) built in DVE 2x mode from a
    pre-expanded dst tensor; M^T (xr-gather lhsT) built with a per-partition
    iota tensor_scalar from a pre-replicated dst row. No PE transposes.
  - alpha kept in bf16: feeds tensor_scalar scaling and the denominator
    matmuls directly (no casts).
  - degree-balanced node relabeling: NT = 17 with zero padding waste.
"""
import os
import numpy as np
import ml_dtypes

import concourse.bacc as bacc
import concourse.bass as bass
import concourse.mybir as mybir
import concourse.tile as tile
from concourse.bass_utils import run_bass_kernel_spmd
from concourse.masks import make_identity

P = 128
N = 16384
NCORES = 8
NLOC = N // NCORES          # 2048
NBLK = NLOC // P            # 16
F_IN = 128
DIM = 64
HID = 256
FP = mybir.dt.float32
BF = mybir.dt.bfloat16
I16 = mybir.dt.int16
AF = mybir.ActivationFunctionType
ALU = mybir.AluOpType
BF_NP = ml_dtypes.bfloat16
NEG = 0.2

LAST_RESULTS = None


def _balance_nodes(edge_index):
    """Permute node ids so every 128-node block has ~equal in-degree sum."""
    import heapq
    deg = np.bincount(edge_index[1], minlength=N).astype(np.int64) + 1  # + self loop
    order = np.argsort(-deg, kind="stable")
    nblk_g = N // P
    heap = [(0, g) for g in range(nblk_g)]
    heapq.heapify(heap)
    counts = np.zeros(nblk_g, np.int64)
    loads = np.zeros(nblk_g, np.int64)
    assign = np.empty(N, np.int64)
    for v in order:
        while True:
            load, g = heapq.heappop(heap)
            if counts[g] < P:
                break
        assign[v] = g
        counts[g] += 1
        loads[g] = load + deg[v]
        if counts[g] < P:
            heapq.heappush(heap, (loads[g], g))
    # repair: shave overloaded blocks down to the ceiling with 1-for-1 swaps
    # of nodes whose degrees differ by exactly the overload delta (usually 1)
    cap = int(np.ceil(deg.sum() / nblk_g))
    by_block = [list(np.where(assign == g)[0]) for g in range(nblk_g)]
    for _ in range(64):
        over = [g for g in range(nblk_g) if loads[g] > cap]
        if not over:
            break
        under = sorted((g for g in range(nblk_g) if loads[g] < cap),
                       key=lambda g: loads[g])
        for go in over:
            d = int(loads[go] - cap)
            done = False
            for gu in under:
                if loads[gu] + d > cap:
                    continue
                du = {deg[v]: v for v in by_block[gu]}
                for a in by_block[go]:
                    b = du.get(deg[a] - d)
                    if b is not None:
                        assign[a], assign[b] = gu, go
                        by_block[go].remove(a); by_block[gu].remove(b)
                        by_block[go].append(b); by_block[gu].append(a)
                        loads[go] -= d; loads[gu] += d
                        done = True
                        break
                if done:
                    break
    perm = np.argsort(assign, kind="stable")      # new -> old
    rank = np.empty(N, np.int64)
    rank[perm] = np.arange(N)                     # old -> new
    return perm, rank


def _prep_edges(src, dst):
    """src/dst already relabeled. Returns gather indices + dst tables."""
    order = np.argsort(dst, kind="stable")
    src_s, dst_s = src[order], dst[order]
    blk = dst_s // P
    bc = np.bincount(blk, minlength=NCORES * NBLK)
    NT = int(np.ceil(bc.max() / P))
    EBLK = NT * P
    src_pad = np.zeros((NCORES, NBLK, EBLK), dtype=np.int32)
    dst_pad = np.full((NCORES, NBLK, EBLK), P, dtype=np.float32)   # P = pad marker
    starts = np.concatenate([[0], np.cumsum(bc)])
    for g in range(NCORES * NBLK):
        c, b = divmod(g, NBLK)
        s, e = starts[g], starts[g + 1]
        k = e - s
        src_pad[c, b, :k] = src_s[s:e]
        dst_pad[c, b, :k] = (dst_s[s:e] - g * P).astype(np.float32)
    # dma_gather idx layout: flat[i] = idxs[i % 16, i // 16], 16-row pattern
    # replicated down 128 partitions
    idx16 = (src_pad.reshape(NCORES, NBLK, EBLK // 16, 16)
             .transpose(0, 1, 3, 2).astype(np.int16))
    idx16 = np.tile(idx16, (1, 1, 8, 1)).copy()        # [NC, NBLK, 128, EBLK/16]
    # slot (p, t) = flat index t*P + p
    dst_col = dst_pad.reshape(NCORES, NBLK, NT, P).transpose(0, 1, 3, 2)
    # dstpack: [P, NT*P + NT*P] =
    #   [dstrep (dst replicated across partitions, for the MT build)
    #    | dst_exp (dst value repeated along d, for the 2x-mode M build)]
    dstpack = np.empty((NCORES, NBLK, P, NT * 2 * P), dtype=BF_NP)
    dstpack[:, :, :, :EBLK] = np.broadcast_to(
        dst_pad[:, :, None, :], (NCORES, NBLK, P, EBLK)).astype(BF_NP)
    dstpack[:, :, :, EBLK:] = np.repeat(dst_col, P, axis=3).astype(BF_NP)
    return idx16, dstpack, NT


def _build(NT):
    nc = bacc.Bacc(None)
    NPAIR = (NT + 1) // 2

    def par(name, shape, dtype=BF):
        return nc.declare_dram_parameter(name, list(shape), dtype, isOutput=False)

    xT = par("xT", [F_IN, NLOC])
    idx16 = par("idx16", [NBLK, P, NT * 8], I16)
    dstpack = par("dstpack", [NBLK, P, NT * 2 * P])
    Win = par("Win", [F_IN, DIM]); b_in = par("b_in", [1, DIM])
    Wskip = par("Wskip", [DIM, HID]); bskip = par("bskip", [1, HID])
    WL1 = par("WL1", [DIM, HID]); WR1 = par("WR1", [DIM, HID])
    WL2 = par("WL2", [HID, HID]); WR2 = par("WR2", [HID, HID])
    WL3 = par("WL3", [HID, HID]); WR3 = par("WR3", [HID, HID])
    attv1 = par("attv1", [P, 4 * HID]); attv2 = par("attv2", [P, 4 * HID])
    attv3 = par("attv3", [P, 4 * HID])
    Wm1 = par("Wm1", [HID, DIM]); bm1 = par("bm1", [1, DIM])
    Wm2 = par("Wm2", [DIM, DIM]); bm2 = par("bm2", [1, DIM])
    Wm3 = par("Wm3", [DIM, 1]); bm3 = par("bm3", [1, 1])
    iota_row = par("iota_row", [P, P])
    iota_col = par("iota_col", [P, 1], FP)
    out = nc.declare_dram_parameter("out", [1, NLOC], FP, isOutput=True)

    xl_loc = {l: nc.dram_tensor(f"xl_loc{l}", [NLOC, HID], BF) for l in (1, 2, 3)}
    xl_full = {l: nc.dram_tensor(f"xl_full{l}", [N, HID], BF, addr_space="Shared")
               for l in (1, 2, 3)}

    with tile.TileContext(nc) as tc:
        with (
            tc.tile_pool(name="const", bufs=1) as cp,
            tc.tile_pool(name="big", bufs=1) as bigp,
            tc.tile_pool(name="wk", bufs=1) as wk,
            tc.tile_pool(name="ps_mm", bufs=2, space="PSUM") as ps_mm,
            tc.tile_pool(name="ps_out", bufs=2, space="PSUM") as ps_out_pool,
            tc.tile_pool(name="ps_w", bufs=2, space="PSUM") as ps_w_pool,
        ):
            def load_const(pname, ap, shape, dtype=BF):
                t = cp.tile(list(shape), dtype, name=pname + "_sb")
                nc.sync.dma_start(out=t[:], in_=ap[:])
                return t

            def load_const_2k(pname, ap, rows, cols):
                assert rows == 2 * P
                t = cp.tile([P, 2 * cols], BF, name=pname + "_sb")
                nc.sync.dma_start(out=t[:, :cols], in_=ap[:P, :])
                nc.sync.dma_start(out=t[:, cols:], in_=ap[P:, :])
                return t

            ident_f = cp.tile([P, P], FP, name="ident_f")
            make_identity(nc, ident_f[:])
            ident_b = cp.tile([P, P], BF, name="ident_b")
            nc.vector.tensor_copy(out=ident_b[:], in_=ident_f[:])
            ones_row = cp.tile([1, 512], BF, name="ones_row")
            nc.vector.memset(ones_row[:], 1.0)
            ones_col = cp.tile([P, 1], BF, name="ones_col")
            nc.vector.memset(ones_col[:], 1.0)

            xT_sb = load_const("xT", xT, [F_IN, NLOC])
            Win_sb = load_const("Win", Win, [F_IN, DIM])
            b_in_sb = load_const("b_in", b_in, [1, DIM])
            Wskip_sb = load_const("Wskip", Wskip, [DIM, HID])
            bskip_sb = load_const("bskip", bskip, [1, HID])
            WL_sb = {1: load_const("WL1", WL1, [DIM, HID]),
                     2: load_const_2k("WL2", WL2, HID, HID),
                     3: load_const_2k("WL3", WL3, HID, HID)}
            WR_sb = {1: load_const("WR1", WR1, [DIM, HID]),
                     2: load_const_2k("WR2", WR2, HID, HID),
                     3: load_const_2k("WR3", WR3, HID, HID)}
            attv_sb = {1: load_const("attv1", attv1, [P, 4 * HID]),
                       2: load_const("attv2", attv2, [P, 4 * HID]),
                       3: load_const("attv3", attv3, [P, 4 * HID])}
            Wm1_sb = load_const_2k("Wm1", Wm1, HID, DIM)
            bm1_sb = load_const("bm1", bm1, [1, DIM])
            Wm2_sb = load_const("Wm2", Wm2, [DIM, DIM])
            bm2_sb = load_const("bm2", bm2, [1, DIM])
            Wm3_sb = load_const("Wm3", Wm3, [DIM, 1])
            bm3_sb = load_const("bm3", bm3, [1, 1])
            ior_sb = load_const("iota_row", iota_row, [P, P])
            ioc_sb = load_const("iota_col", iota_col, [P, 1], FP)

            actT = {0: bigp.tile([P, NLOC], BF, name="actT0"),
                    1: bigp.tile([P, NLOC], BF, name="actT1")}
            act_prev = bigp.tile([P, NBLK * HID], FP, name="act_prev")
            act_next = bigp.tile([P, NBLK * HID], FP, name="act_next")
            XRb = bigp.tile([P, NBLK * HID], BF, name="XRb")
            hT = bigp.tile([DIM, NLOC], BF, name="hT")
            m1T = bigp.tile([DIM, NLOC], BF, name="m1T")
            m2T = bigp.tile([DIM, NLOC], BF, name="m2T")
            y_sb = bigp.tile([1, NLOC], FP, name="y_sb")

            # ---------------- phase A: h = relu(x @ Win + b) ----------------
            for j in range(NLOC // 512):
                sl = slice(j * 512, (j + 1) * 512)
                pmm = ps_mm.tile([P, 512], FP, space="PSUM", name="pmm", tag="pmm")
                nc.tensor.matmul(out=pmm[:DIM, :], lhsT=Win_sb[:], rhs=xT_sb[:, sl],
                                 start=True, stop=False)
                nc.tensor.matmul(out=pmm[:DIM, :], lhsT=b_in_sb[:], rhs=ones_row[:],
                                 start=False, stop=True)
                nc.scalar.activation(out=hT[:DIM, sl], in_=pmm[:DIM, :], func=AF.Relu)

            # layer-1 XL/XR + skip
            for b in range(NBLK):
                nsl = slice(b * P, (b + 1) * P)
                pxl = ps_mm.tile([P, HID], FP, space="PSUM", name="pxl", tag="pmm")
                nc.tensor.matmul(out=pxl[:], lhsT=hT[:DIM, nsl], rhs=WL_sb[1][:],
                                 start=True, stop=True)
                xl_st = wk.tile([P, HID], BF, name="xl_st", tag="xl_st", bufs=3)
                nc.scalar.activation(out=xl_st[:], in_=pxl[:], func=AF.Copy)
                nc.sync.dma_start(out=xl_loc[1][nsl, :], in_=xl_st[:])

                pxr = ps_mm.tile([P, HID], FP, space="PSUM", name="pxr", tag="pmm")
                nc.tensor.matmul(out=pxr[:], lhsT=hT[:DIM, nsl], rhs=WR_sb[1][:],
                                 start=True, stop=True)
                nc.scalar.activation(out=XRb[:, b * HID:(b + 1) * HID], in_=pxr[:],
                                     func=AF.Copy)

                psk = ps_mm.tile([P, HID], FP, space="PSUM", name="psk", tag="pmm")
                nc.tensor.matmul(out=psk[:], lhsT=hT[:DIM, nsl], rhs=Wskip_sb[:],
                                 start=True, stop=False)
                nc.tensor.matmul(out=psk[:], lhsT=ones_row[:, :P], rhs=bskip_sb[:],
                                 start=False, stop=True)
                nc.scalar.activation(out=act_prev[:, b * HID:(b + 1) * HID], in_=psk[:],
                                     func=AF.Copy)

            nc.gpsimd.collective_compute(
                "AllGather", ALU.bypass, replica_groups=[list(range(NCORES))],
                ins=[xl_loc[1][:]], outs=[xl_full[1][:]])

            # ---------------- edge stage ----------------
            def edge_layer(l, H):
                D = HID // H
                for b in range(NBLK):
                    idx_sb = wk.tile([P, NT * 8], I16, name="idx_sb",
                                     tag="idx_sb", bufs=3)
                    nc.sync.dma_start(out=idx_sb[:], in_=idx16[b])
                    dpk = wk.tile([P, NT * 2 * P], BF, name="dpk", tag="dpk", bufs=3)
                    nc.sync.dma_start(out=dpk[:], in_=dstpack[b])
                    drep = dpk[:, :NT * P]
                    dexp = dpk[:, NT * P:]
                    # edge-major one-hot M[e, (t,d)]  (both operands unit-stride)
                    M = wk.tile([P, NT * P], BF, name="M", tag="M", bufs=3)
                    nc.vector.tensor_tensor(
                        out=M[:].rearrange("p (t e) -> p t e", t=NT),
                        in0=dexp[:].rearrange("p (t e) -> p t e", t=NT),
                        in1=ior_sb[:, None, :].to_broadcast([P, NT, P]),
                        op=ALU.is_equal)
                    # dst-major one-hot MT[d, (t,e)]
                    MT = wk.tile([P, NT * P], BF, name="MT", tag="MT", bufs=3)
                    nc.vector.tensor_scalar(out=MT[:], in0=drep,
                                            scalar1=ioc_sb[:, :1], scalar2=None,
                                            op0=ALU.is_equal)
                    xl_all = wk.tile([P, NT * HID], BF, name="xl_all", tag="xl_all",
                                     bufs=3)
                    nc.gpsimd.dma_gather(
                        out_ap=xl_all[:].rearrange("p (t c) -> p t c", t=NT),
                        in_ap=xl_full[l][:],
                        idxs_ap=idx_sb[:],
                        num_idxs=NT * P, num_idxs_reg=NT * P,
                        elem_size=HID, single_packet=False)

                    r_all = wk.tile([P, NT * H], FP, name="r_all", tag="r_all", bufs=2)
                    # process 4 t-tiles per 2-bank PSUM group: 1 Prelu + 1 mult
                    # + 1 segmented reduce per quad
                    for q in range((NT + 3) // 4):
                        t0 = 4 * q
                        wq = min(4, NT - t0)
                        W = wq * HID
                        wps = ps_w_pool.tile([P, 1024], FP, space="PSUM",
                                             name="wps", tag="wps")
                        for i in range(wq):
                            # start=True only on the first matmul touching each
                            # PSUM bank (the start flag clears the whole bank)
                            nc.tensor.matmul(
                                out=wps[:, i * HID:(i + 1) * HID],
                                lhsT=MT[:, (t0 + i) * P:(t0 + i + 1) * P],
                                rhs=XRb[:, b * HID:(b + 1) * HID],
                                start=(i == 0 or i == 2), stop=False)
                        for hf in range((wq + 1) // 2):
                            w0 = hf * 512
                            w1 = min(W, w0 + 512)
                            nc.tensor.matmul(out=wps[:, w0:w1], lhsT=ident_b[:],
                                             rhs=xl_all[:, t0 * HID + w0:
                                                        t0 * HID + w1],
                                             start=False, stop=True)
                        e_q = wk.tile([P, 1024], BF, name="e_q", tag="e_q", bufs=3)
                        nc.scalar.activation(out=e_q[:, :W], in_=wps[:, :W],
                                             func=AF.Prelu, alpha=NEG)
                        z_q = wk.tile([P, 1024], BF, name="z_q", tag="z_q", bufs=3)
                        nc.vector.tensor_tensor(out=z_q[:, :W], in0=e_q[:, :W],
                                                in1=attv_sb[l][:, :W], op=ALU.mult)
                        nc.vector.tensor_reduce(
                            out=r_all[:, t0 * H:(t0 + wq) * H],
                            in_=z_q[:, :W].rearrange("p (u d) -> p u d", d=D),
                            axis=mybir.AxisListType.X, op=ALU.add)

                    alpha_all = wk.tile([P, NT * H], FP, name="alpha_all",
                                        tag="alpha_all", bufs=2)
                    nc.scalar.activation(out=alpha_all[:], in_=r_all[:], func=AF.Exp)
                    if H > 1:
                        alpha_bf = wk.tile([P, NT * H], BF, name="alpha_bf",
                                           tag="alpha_bf", bufs=2)
                        nc.vector.tensor_copy(out=alpha_bf[:], in_=alpha_all[:])

                    ps_o = ps_out_pool.tile([P, 264], FP, space="PSUM",
                                            name="ps_o", tag="ps_o")
                    if H == 1:
                        for t in range(NT):
                            mp = wk.tile([P, P], BF, name="mp", tag="mp", bufs=4)
                            if t % 2 == 0:
                                nc.vector.tensor_scalar(
                                    out=mp[:], in0=M[:, t * P:(t + 1) * P],
                                    scalar1=alpha_all[:, t:t + 1], scalar2=None,
                                    op0=ALU.mult)
                            else:
                                nc.scalar.activation(
                                    out=mp[:], in_=M[:, t * P:(t + 1) * P],
                                    func=AF.Copy, scale=alpha_all[:, t:t + 1])
                            nc.tensor.matmul(out=ps_o[:, :HID], lhsT=mp[:],
                                             rhs=xl_all[:, t * HID:(t + 1) * HID],
                                             start=(t == 0), stop=(t == NT - 1))
                            nc.tensor.matmul(out=ps_o[:, HID:HID + 1], lhsT=mp[:],
                                             rhs=ones_col[:],
                                             start=False, stop=(t == NT - 1))
                    else:
                        for tt in range(0, NT, 2):
                            wv = min(2, NT - tt)
                            v2 = wk.tile([P, 2 * HID], BF, name="v2", tag="v2",
                                         bufs=3)
                            nc.vector.tensor_tensor(
                                out=v2[:, :wv * HID].rearrange(
                                    "p (u d) -> p u d", d=D),
                                in0=xl_all[:, tt * HID:(tt + wv) * HID].rearrange(
                                    "p (u d) -> p u d", d=D),
                                in1=alpha_bf[:, tt * H:(tt + wv) * H, None
                                             ].to_broadcast([P, wv * H, D]),
                                op=ALU.mult)
                            for i in range(wv):
                                t = tt + i
                                nc.tensor.matmul(out=ps_o[:, :HID],
                                                 lhsT=M[:, t * P:(t + 1) * P],
                                                 rhs=v2[:, i * HID:(i + 1) * HID],
                                                 start=(t == 0),
                                                 stop=(t == NT - 1))
                                nc.tensor.matmul(out=ps_o[:, HID:HID + H],
                                                 lhsT=M[:, t * P:(t + 1) * P],
                                                 rhs=alpha_bf[:, t * H:(t + 1) * H],
                                                 start=False, stop=(t == NT - 1))
                    # finalize
                    rec = wk.tile([P, H], FP, name="rec", tag="rec", bufs=2)
                    nc.vector.reciprocal(out=rec[:], in_=ps_o[:, HID:HID + H])
                    g_t = wk.tile([P, HID], FP, name="g_t", tag="g_t", bufs=2)
                    for h in range(H):
                        hs = slice(h * D, (h + 1) * D)
                        nc.vector.tensor_scalar(out=g_t[:, hs], in0=ps_o[:, hs],
                                                scalar1=rec[:, h:h + 1], scalar2=None,
                                                op0=ALU.mult)
                    gr = wk.tile([P, HID], BF, name="gr", tag="gr", bufs=2)
                    nc.scalar.activation(out=gr[:], in_=g_t[:], func=AF.Relu)
                    bsl = slice(b * HID, (b + 1) * HID)
                    nc.vector.tensor_tensor(out=act_next[:, bsl], in0=gr[:],
                                            in1=act_prev[:, bsl], op=ALU.add)
                    for k in range(2):
                        tp = ps_mm.tile([P, P], FP, space="PSUM", name="tp", tag="pmm")
                        nc.tensor.transpose(
                            out=tp[:],
                            in_=act_next[:, b * HID + k * P:b * HID + (k + 1) * P],
                            identity=ident_f[:])
                        nc.scalar.activation(out=actT[k][:, b * P:(b + 1) * P],
                                             in_=tp[:], func=AF.Copy)

            def xlxr_layer(l):
                for b in range(NBLK):
                    nsl = slice(b * P, (b + 1) * P)
                    pxl = ps_mm.tile([P, HID], FP, space="PSUM", name="pxl2", tag="pmm")
                    for k in range(2):
                        nc.tensor.matmul(out=pxl[:], lhsT=actT[k][:, nsl],
                                         rhs=WL_sb[l][:, k * HID:(k + 1) * HID],
                                         start=(k == 0), stop=(k == 1))
                    xl_st = wk.tile([P, HID], BF, name="xl_st2", tag="xl_st", bufs=3)
                    nc.scalar.activation(out=xl_st[:], in_=pxl[:], func=AF.Copy)
                    nc.sync.dma_start(out=xl_loc[l][nsl, :], in_=xl_st[:])

                    pxr = ps_mm.tile([P, HID], FP, space="PSUM", name="pxr2", tag="pmm")
                    for k in range(2):
                        nc.tensor.matmul(out=pxr[:], lhsT=actT[k][:, nsl],
                                         rhs=WR_sb[l][:, k * HID:(k + 1) * HID],
                                         start=(k == 0), stop=(k == 1))
                    nc.scalar.activation(out=XRb[:, b * HID:(b + 1) * HID], in_=pxr[:],
                                         func=AF.Copy)
                nc.gpsimd.collective_compute(
                    "AllGather", ALU.bypass, replica_groups=[list(range(NCORES))],
                    ins=[xl_loc[l][:]], outs=[xl_full[l][:]])

            edge_layer(1, 4)
            act_prev, act_next = act_next, act_prev
            xlxr_layer(2)
            edge_layer(2, 1)
            act_prev, act_next = act_next, act_prev
            xlxr_layer(3)
            edge_layer(3, 1)

            # ---------------- MLP head ----------------
            for j in range(NLOC // 512):
                sl = slice(j * 512, (j + 1) * 512)
                pm1 = ps_mm.tile([P, 512], FP, space="PSUM", name="pm1", tag="pmm")
                for k in range(2):
                    nc.tensor.matmul(out=pm1[:DIM, :],
                                     lhsT=Wm1_sb[:, k * DIM:(k + 1) * DIM],
                                     rhs=actT[k][:, sl], start=(k == 0), stop=False)
                nc.tensor.matmul(out=pm1[:DIM, :], lhsT=bm1_sb[:], rhs=ones_row[:],
                                 start=False, stop=True)
                nc.scalar.activation(out=m1T[:DIM, sl], in_=pm1[:DIM, :], func=AF.Relu)
            for j in range(NLOC // 512):
                sl = slice(j * 512, (j + 1) * 512)
                pm2 = ps_mm.tile([P, 512], FP, space="PSUM", name="pm2", tag="pmm")
                nc.tensor.matmul(out=pm2[:DIM, :], lhsT=Wm2_sb[:], rhs=m1T[:DIM, sl],
                                 start=True, stop=False)
                nc.tensor.matmul(out=pm2[:DIM, :], lhsT=bm2_sb[:], rhs=ones_row[:],
                                 start=False, stop=True)
                nc.scalar.activation(out=m2T[:DIM, sl], in_=pm2[:DIM, :], func=AF.Relu)
            for j in range(NLOC // 512):
                sl = slice(j * 512, (j + 1) * 512)
                py = ps_mm.tile([P, 512], FP, space="PSUM", name="py", tag="pmm")
                nc.tensor.matmul(out=py[:1, :], lhsT=Wm3_sb[:], rhs=m2T[:DIM, sl],
                                 start=True, stop=False)
                nc.tensor.matmul(out=py[:1, :], lhsT=bm3_sb[:], rhs=ones_row[:],
                                 start=False, stop=True)
                nc.scalar.activation(out=y_sb[:, sl], in_=py[:1, :], func=AF.Copy)
            nc.sync.dma_start(out=out[:], in_=y_sb[:])

    nc.compile()
    return nc


_BUILD_CACHE = {}


def _get_program(NT):
    if NT not in _BUILD_CACHE:
        _BUILD_CACHE[NT] = _build(NT)
    return _BUILD_CACHE[NT]


def kernel(**inputs) -> np.ndarray:
    global LAST_RESULTS
    ii = {k: np.asarray(v) for k, v in inputs.items()}
    assert ii["x"].shape == (N, F_IN)
    for l in (1, 2, 3):
        assert not np.any(ii[f"b{l}"]), "GAT bias assumed zero"

    ei = np.asarray(ii["edge_index"], np.int64)
    perm, rank = _balance_nodes(ei)
    src = np.concatenate([rank[ei[0]], np.arange(N, dtype=np.int64)])
    dst = np.concatenate([rank[ei[1]], np.arange(N, dtype=np.int64)])
    idx16, dstpack, NT = _prep_edges(src, dst)

    def bf(a):
        return np.asarray(a, np.float32).astype(BF_NP)

    att_row = {}
    for l in (1, 2, 3):
        a = np.asarray(ii[f"att{l}"], np.float32).reshape(-1)  # [HID]
        att_row[l] = np.tile(np.concatenate([a, a, a, a])[None, :],
                             (P, 1)).astype(BF_NP)

    iota = np.arange(P, dtype=np.float32)
    common = dict(
        Win=bf(ii["Win"]), b_in=bf(ii["b_in"])[None, :],
        Wskip=bf(ii["Wskip"]), bskip=bf(ii["bskip"])[None, :],
        WL1=bf(ii["Wl1"]), WR1=bf(ii["Wr1"]), attv1=att_row[1],
        WL2=bf(ii["Wl2"]), WR2=bf(ii["Wr2"]), attv2=att_row[2],
        WL3=bf(ii["Wl3"]), WR3=bf(ii["Wr3"]), attv3=att_row[3],
        Wm1=bf(ii["Wm1"]), bm1=bf(ii["bm1"])[None, :],
        Wm2=bf(ii["Wm2"]), bm2=bf(ii["bm2"])[None, :],
        Wm3=bf(ii["Wm3"]), bm3=bf(ii["bm3"])[None, :],
        iota_row=np.tile(iota[None, :], (P, 1)).astype(BF_NP),
        iota_col=iota[:, None].copy(),
    )
    x = np.asarray(ii["x"], np.float32)[perm]
    in_maps = []
    for c in range(NCORES):
        m = dict(common)
        m["xT"] = np.ascontiguousarray(x[c * NLOC:(c + 1) * NLOC].T).astype(BF_NP)
        m["idx16"] = idx16[c]
        m["dstpack"] = dstpack[c]
        in_maps.append(m)

    nc = _get_program(NT)
    res = run_bass_kernel_spmd(nc, in_maps, list(range(NCORES)),
                               trace=bool(os.environ.get("GAT_TRACE")))
    LAST_RESULTS = res
    y_new = np.concatenate([res.results[c]["out"].reshape(-1) for c in range(NCORES)])
    y = np.empty(N, np.float32)
    y[perm] = y_new
    return y
